# revision 5
# baseline (speedup 1.0000x reference)
"""Trainium2 Bass kernel for nn_BinaryDense (binary-masked dense layer).

Computes out = x @ mask where
  p    = sigmoid(M)          (bit-exact neuron lowering: exp(-x), +1, recip)
  bern = (u < p)
  mask = (2*bern - 1) * STD,  STD = 1/64 (exactly representable in fp8 e4m3)

Sharding: column-shard M/u/units 8 ways (512 cols per core); every core
consumes the full x and produces out[:, 512*i : 512*(i+1)].

Matmul: fp8e4 DoubleRow perf mode — each matmul contracts TWO 128-k slabs
(lhsT [128k, 2, 128m] stationary, rhs = mask [128k, 2, 512n] moving) at
0.5 cycles/row, 4x the fp16 rate in the TimelineSim cost model. x is split
x = hi + lo (both e4m3) on the host; hi and lo passes accumulate into the
same PSUM bank, recovering ~fp16 accuracy (mask values are exact in fp8).

x layout: [MPAIRS, K, 512] fp8 where cols 0:256 hold the hi of an m-pair
(256 rows) and cols 256:512 the lo; 512B DMA inner runs (full modeled DMA
bandwidth; <512B runs pay 2x).

Out is written fp16 (halves out DMA; keeps the steady state PE-bound) and
converted to fp32 on the host.

Head interleave: while mask groups are produced (DMA-bound), the first
HEADP m-tile-pairs accumulate the already-available groups across PSUM
banks, keeping the PE busy through the mask phase.
"""

import os
import numpy as np
import ml_dtypes

import concourse.bass as bass
import concourse.mybir as mybir
import concourse.tile as tile
from concourse import bacc
from concourse.bass_utils import run_bass_kernel_spmd

B = 8192  # x rows
K = 4096  # contraction dim (IN_DIM)
N = 4096  # units
STD = 1.0 / 64.0

NCORES = 8
NSHARD = N // NCORES  # 512 output cols per core
KSLABS = K // 128  # 32
MTILES = B // 128  # 64
MPAIRS = MTILES // 2  # 32
NSUB = NSHARD  # moving free dim per matmul (<=512 fp32 psum bank)

# mask group k-slab ranges (uniform pairs)
GROUPS = [(s, 2) for s in range(0, KSLABS, 2)]
NGRP = len(GROUPS)

F8 = mybir.dt.float8e4
F8NP = ml_dtypes.float8_e4m3

MODE = os.environ.get("BINARYDENSE_MODE", "fp8dr")


def build_nc(mode: str, headp: int = 3):
    assert mode == "fp8dr"
    DR = mybir.MatmulPerfMode.DoubleRow

    nc = bacc.Bacc(
        "TRN2", target_bir_lowering=False, debug=False, num_devices=NCORES
    )
    xt8 = nc.declare_dram_parameter("xt8", [MPAIRS, K, 512], F8, isOutput=False)
    m_in = nc.declare_dram_parameter(
        "m_in", [K, NSHARD], mybir.dt.float32, isOutput=False
    )
    u_in = nc.declare_dram_parameter(
        "u_in", [K, NSHARD], mybir.dt.float32, isOutput=False
    )
    out = nc.declare_dram_parameter(
        "out", [B, NSHARD], mybir.dt.float16, isOutput=True
    )

    with tile.TileContext(nc) as tc:
        with (
            tc.tile_pool(name="mask", bufs=1) as mask_pool,
            tc.tile_pool(name="maskwork", bufs=2) as work_pool,
            tc.tile_pool(name="xt", bufs=3) as xt_pool,
            tc.tile_pool(name="xthead", bufs=1) as xt_head_pool,
            tc.tile_pool(name="outcp", bufs=3) as out_pool,
            tc.tile_pool(name="psum", bufs=1, space="PSUM") as psum_pool,
        ):
            mask_groups = []

            def make_mask_group(g):
                """Emit mask production for group g (fp8 output)."""
                s0, cnt = GROUPS[g]
                gw = cnt * NSHARD
                r = s0 * 128
                m_t = work_pool.tile([128, gw], mybir.dt.float32, name=f"m_t{cnt}")
                nc.gpsimd.dma_start(
                    out=m_t.rearrange("p (s n) -> p s n", s=cnt),
                    in_=m_in[r : r + cnt * 128, :].rearrange("(s p) n -> p s n", p=128),
                )
                u_t = work_pool.tile([128, gw], mybir.dt.float32, name=f"u_t{cnt}")
                nc.gpsimd.dma_start(
                    out=u_t.rearrange("p (s n) -> p s n", s=cnt),
                    in_=u_in[r : r + cnt * 128, :].rearrange("(s p) n -> p s n", p=128),
                )
                # p = 1/(1+exp(-m)) -- must match neuron's logistic lowering
                # bit-exactly (ACT Exp table, fp32 add, DVE reciprocal).
                ex = work_pool.tile([128, gw], mybir.dt.float32, name=f"ex{cnt}")
                nc.scalar.activation(
                    ex, m_t, mybir.ActivationFunctionType.Exp, scale=-1.0
                )
                den = work_pool.tile([128, gw], mybir.dt.float32, name=f"den{cnt}")
                nc.vector.tensor_scalar(
                    out=den, in0=ex, scalar1=1.0, scalar2=None,
                    op0=mybir.AluOpType.add,
                )
                p_t = work_pool.tile([128, gw], mybir.dt.float32, name=f"p_t{cnt}")
                nc.vector.reciprocal(p_t, den)
                bern = work_pool.tile([128, gw], F8, name=f"bern{cnt}")
                nc.vector.tensor_tensor(
                    out=bern, in0=u_t, in1=p_t, op=mybir.AluOpType.is_lt
                )
                mk = mask_pool.tile([128, gw], F8, name=f"mask{g}")
                nc.vector.tensor_scalar(
                    out=mk, in0=bern, scalar1=2.0 * STD, scalar2=-STD,
                    op0=mybir.AluOpType.mult, op1=mybir.AluOpType.add,
                )
                mask_groups.append(mk)

            def load_pair(mp, pool, name):
                """Load m-pair mp: SBUF [128, KSLABS*512] fp8 (s-major,
                within slab: cols 0:256 hi, 256:512 lo)."""
                xh = pool.tile([128, K * 4], F8, name=name)
                nc.sync.dma_start(
                    out=xh.rearrange("p (s c) -> p s c", s=KSLABS),
                    in_=xt8[mp].rearrange("(s p) c -> p s c", p=128),
                )
                return xh

            def mm_group(ps, xv, half, g, first, last):
                """Emit hi+lo DoubleRow matmuls of slab-pair group g for
                m-tile (pair, half) into psum ps."""
                rhs = mask_groups[g].rearrange("p (s n) -> p s n", s=2)
                nc.tensor.matmul(
                    ps,
                    lhsT=xv[:, 2 * g : 2 * g + 2, half * 128 : half * 128 + 128],
                    rhs=rhs,
                    start=first,
                    stop=False,
                    perf_mode=DR,
                )
                nc.tensor.matmul(
                    ps,
                    lhsT=xv[:, 2 * g : 2 * g + 2, 256 + half * 128 : 256 + half * 128 + 128],
                    rhs=rhs,
                    start=False,
                    stop=last,
                    perf_mode=DR,
                )

            def store_out(mt, ps):
                o_t = out_pool.tile([128, NSUB], mybir.dt.float16)
                nc.vector.tensor_copy(o_t, ps)
                nc.scalar.dma_start(out=out[mt * 128 : (mt + 1) * 128, :], in_=o_t)

            # ---- Head: interleave mask production with first pairs ----
            # Emission order matters: group g's mask DMA must precede pair
            # g's xt load so mask production is never queued behind x data.
            head = []

            def add_head_pair(mp):
                xh = load_pair(mp, xt_head_pool, f"xthead{mp}")
                xv = xh.rearrange("p (s c) -> p s c", s=KSLABS)
                ps0 = psum_pool.tile(
                    [128, NSUB], mybir.dt.float32, name=f"psh{mp}a", bufs=1
                )
                ps1 = psum_pool.tile(
                    [128, NSUB], mybir.dt.float32, name=f"psh{mp}b", bufs=1
                )
                head.append((xv, ps0, ps1))

            for g in range(NGRP):
                make_mask_group(g)
                if g < headp:
                    add_head_pair(g)
                # catch-up: pair mp joins at group mp and replays all
                # groups produced so far
                for mp in range(min(g + 1, headp)):
                    xv, ps0, ps1 = head[mp]
                    todo = list(range(g + 1)) if mp == g else [g]
                    for j, gg in enumerate(todo):
                        mm_group(ps0, xv, 0, gg,
                                 first=(mp == g and j == 0),
                                 last=(g == NGRP - 1))
                        mm_group(ps1, xv, 1, gg,
                                 first=(mp == g and j == 0),
                                 last=(g == NGRP - 1))
            for mp in range(headp):
                store_out(2 * mp, head[mp][1])
                store_out(2 * mp + 1, head[mp][2])

            # ---- Steady state: remaining pairs ----
            for mp in range(headp, MPAIRS):
                xh = load_pair(mp, xt_pool, "xh")
                xv = xh.rearrange("p (s c) -> p s c", s=KSLABS)
                for half in range(2):
                    ps = psum_pool.tile(
                        [128, NSUB], mybir.dt.float32, name="ps", bufs=2
                    )
                    for g in range(NGRP):
                        mm_group(ps, xv, half, g,
                                 first=(g == 0), last=(g == NGRP - 1))
                    store_out(2 * mp + half, ps)

    nc.finalize()
    return nc


_NC_CACHE: dict[str, object] = {}


def _get_nc(mode: str):
    if mode not in _NC_CACHE:
        _NC_CACHE[mode] = build_nc(mode)
    return _NC_CACHE[mode]


def _prep_inputs(x, M, u, mode: str):
    xT = np.ascontiguousarray(x.T)  # [K, B] f32
    # [MPAIRS, K, 256] f32 blocks (m-pairs of 256 rows)
    blocked = np.ascontiguousarray(
        xT.reshape(K, MPAIRS, 256).transpose(1, 0, 2)
    )
    hi = blocked.astype(F8NP)
    lo = (blocked - hi.astype(np.float32)).astype(F8NP)
    xt8 = np.empty((MPAIRS, K, 512), dtype=F8NP)
    xt8[:, :, 0:256] = hi
    xt8[:, :, 256:512] = lo

    in_maps = []
    for i in range(NCORES):
        cs = slice(i * NSHARD, (i + 1) * NSHARD)
        in_maps.append({
            "xt8": xt8,
            "m_in": np.ascontiguousarray(M[:, cs]),
            "u_in": np.ascontiguousarray(u[:, cs]),
        })
    return in_maps


def run(x, M, u, mode: str | None = None, trace: bool = False):
    mode = mode or MODE
    nc = _get_nc(mode)
    in_maps = _prep_inputs(x, M, u, mode)
    res = run_bass_kernel_spmd(nc, in_maps, list(range(NCORES)), trace=trace)
    out = np.concatenate(
        [res.results[i]["out"].astype(np.float32) for i in range(NCORES)], axis=1
    )
    return out, res


def kernel(x, M, u):
    out, _ = run(np.asarray(x), np.asarray(M), np.asarray(u))
    return out
